# revision 7
# baseline (speedup 1.0000x reference)
"""Multi-head causal attention on 8 Trainium2 NeuronCores.

Sharding: core c handles batch b=c//4, head group g=c%4 (4 heads of 16).
Per-core Bass kernel computes QKV projection, causal attention in a
transposed-scores layout, and the out-projection partial; the host sums
the 4 per-batch bf16 partials (the out_proj all-reduce) in fp32 + bias.

v3: fully software-pipelined single pass (vs v2's phase-sequential form).
  - DMA: (wa_o, xt_o block0) pieces first so the QK projection of token
    block 0 streams in a staircase behind the DMA; the remaining token
    blocks land as one instruction each while attention runs.
  - Schedule: windows (p, j) run in j-major order; QK/V of block n+1 and
    the out-projection of block n-1 are spliced into window (., n) as PE
    filler, so the Tensor engine never waits on the ACT exp stream and
    the ACT exp stream starts ~17us into the kernel instead of ~45us.
  - Causal masks run on the (otherwise idle) GpSimd engine so DVE is free
    at window boundaries to evacuate ctx PSUM immediately.
  - Window tail: the two ctx PSUM banks are evacuated with TWO merged
    casts into a scratch tile (den rows ride along: even den at
    scr[64, 0:512], odd den at scr[32, 512:1024]); the banks recycle
    ~1.2us after the last ctx matmul.  The denominator broadcast
    (K=1 ones-matmuls), reciprocal, and the scr->ctxT normalize muls are
    deferred into the NEXT window as filler, off the critical path.
  - PSUM budget: scores 2x[128,1024] (4 banks) + ctx pcA/pcB ring (2) +
    filler/den ring (2) = 8 banks exactly.

Per-core layouts (S=2048 tokens, D=1024, 4 heads x dh=64):
  - qt/kt [128, pair, S] bf16: partitions 0:64 = even head dh, 64:128 =
    odd head dh.
  - scores: two heads of a pair as CONCURRENT row-tiled K=64 matmuls
    into the two banks of one [128, 1024] PSUM tile.
  - ctx: per head one matmul accumulating over k-chunks; a ones-column
    folded into the V stationary gives the softmax denominator for free:
      even head: lhsT = [v(64) | 1] (M=65)  -> ctx rows 0:64, den row 64
      odd head:  lhsT = [0(32)|1|0(31)|v(64)] (M=128) -> den row 32,
                 ctx rows 64:128 (lane-aligned with ctxT's B half).
  - out^T partial [D, S] bf16 = wo.T @ ctxT accumulated over the 2 pairs.
"""

import sys

sys.path.insert(0, "/opt/trn_rl_repo")

import numpy as np
import ml_dtypes

import concourse.bass as bass
import concourse.tile as tile
from concourse import bacc, mybir
from concourse import bass_utils

BF16 = ml_dtypes.bfloat16
F32 = mybir.dt.float32
BF = mybir.dt.bfloat16

N_CORES = 8
S = 2048          # tokens
D = 1024          # model dim
DHC = 256         # head dims per core (4 heads x 64)
DH = 64
NB = 4            # token blocks of 512
NK = 16           # k chunks of 128
NO = 8            # d_in / d_out chunks of 128

_NC_CACHE = None


def _build_core_kernel():
    nc = bacc.Bacc("TRN2", target_bir_lowering=False, debug=False,
                   num_devices=N_CORES)
    xT = nc.dram_tensor("xT", [D, S], BF, kind="ExternalInput").ap()
    w_all = nc.dram_tensor("w_all", [D, 3 * DHC], BF, kind="ExternalInput").ap()
    wo = nc.dram_tensor("wo", [DHC, D], BF, kind="ExternalInput").ap()
    masks = nc.dram_tensor("masks", [128, 128], BF, kind="ExternalInput").ap()
    outT = nc.dram_tensor("outT", [D, S], BF, kind="ExternalOutput").ap()

    with tile.TileContext(nc) as tc:
        _emit(tc, xT, w_all, wo, masks, outT)
    nc.compile()
    return nc


class Stream:
    """Ordered queue of single-step generators consumed as PE filler."""

    def __init__(self):
        self.gens = []

    def push(self, g):
        self.gens.append(g)

    def push_front(self, g):
        self.gens.insert(0, g)

    def take(self, k):
        while k > 0 and self.gens:
            try:
                next(self.gens[0])
                k -= 1
            except StopIteration:
                self.gens.pop(0)

    def drain(self):
        while self.gens:
            try:
                next(self.gens[0])
            except StopIteration:
                self.gens.pop(0)


def _emit(tc, xT, w_all, wo, masks, outT):
    nc = tc.nc
    EXPF = mybir.ActivationFunctionType.Exp

    from contextlib import ExitStack
    ctx = ExitStack()
    const = ctx.enter_context(tc.tile_pool(name="const", bufs=1))
    scrp = ctx.enter_context(tc.tile_pool(name="scrp", bufs=3))
    work = ctx.enter_context(tc.tile_pool(name="work", bufs=3))
    recp = ctx.enter_context(tc.tile_pool(name="recp", bufs=2))
    outp = ctx.enter_context(tc.tile_pool(name="outp", bufs=3))
    ps_s = ctx.enter_context(tc.tile_pool(name="ps_s", bufs=2, space="PSUM"))
    ps_c = ctx.enter_context(tc.tile_pool(name="ps_c", bufs=2, space="PSUM"))
    ps_x = ctx.enter_context(tc.tile_pool(name="ps_x", bufs=2, space="PSUM"))

    # ---- persistent SBUF tensors ----
    xt = const.tile([128, NO, S], BF, tag="xt")          # x^T, d_in chunks
    wa = const.tile([128, NO, 3 * DHC], BF, tag="wa")    # [Wq|Wk|Wv] slices
    wos = const.tile([128, 2, D], BF, tag="wos")         # Wo row chunks
    msk = const.tile([128, 128], BF, tag="msk")          # causal staircase
    qt = const.tile([128, 2, S], BF, tag="qt")           # q^T per pair
    kt = const.tile([128, 2, S], BF, tag="kt")           # k^T per pair
    # v + folded ones columns (see module docstring)
    vsb = const.tile([128, NK, 4, 128], BF, tag="vsb")
    ctxT = const.tile([128, 2, S], BF, tag="ctxT")
    ones = const.tile([128, DH], BF, tag="ones")

    # ---- DMAs: block-0 staircase first, the rest as whole-block strides ----
    wao = w_all.rearrange("(o p) f -> o p f", p=128)
    xTo = xT.rearrange("(o p) s -> o p s", p=128)
    xTp = xT.rearrange("(o p) s -> p o s", p=128)
    for o in range(NO):
        nc.sync.dma_start(wa[:, o, :], wao[o])
        nc.sync.dma_start(xt[:, o, 0:512], xTo[o, :, 0:512])
    nc.sync.dma_start(msk[:], masks)
    nc.sync.dma_start(xt[:, :, 512:1024], xTp[:, :, 512:1024])
    nc.sync.dma_start(wos[:], wo.rearrange("(c p) f -> p c f", p=128))
    nc.sync.dma_start(xt[:, :, 1024:1536], xTp[:, :, 1024:1536])
    nc.sync.dma_start(xt[:, :, 1536:2048], xTp[:, :, 1536:2048])

    nc.vector.memset(ones[:], 1.0)
    # odd-head slots: zero cols 0:64, ones col 32 (denominator row source);
    # even-head slots: ones col 64. Cols 65:128 of even slots stay garbage
    # (never read: even lhsT slice is [:, 0:65]).
    nc.vector.memset(vsb[:, :, 1::2, 0:DH], 0.0)
    nc.vector.memset(vsb[:, :, 0::2, DH], 1.0)
    nc.vector.memset(vsb[:, :, 1::2, 32], 1.0)

    # ---- QK projection of one token block ----
    def qk_block0():
        # block 0 runs standalone in the prologue, o-outer so the matmuls
        # stream behind the (wa_o, xt_o) DMA staircase; uses the scores
        # PSUM pool (scores haven't started yet).
        pq01 = ps_s.tile([128, 1024], F32, tag="ps")
        pq23 = ps_s.tile([128, 1024], F32, tag="ps")
        tiles = [pq01[:, 0:512], pq01[:, 512:1024],
                 pq23[:, 0:512], pq23[:, 512:1024]]
        for o in range(NO):
            for m in range(4):
                nc.tensor.matmul(
                    tiles[m], lhsT=wa[:, o, 128 * m:128 * m + 128],
                    rhs=xt[:, o, 0:512],
                    start=(o == 0), stop=(o == NO - 1),
                    skip_group_check=True)
        for m in range(4):
            dst = qt[:, m, 0:512] if m < 2 else kt[:, m - 2, 0:512]
            nc.scalar.copy(dst, tiles[m])

    def gen_qk(n):
        # QK projection of token block n as a filler stream (one PSUM
        # accumulator at a time; casts on ACT where there's slack).
        n_sl = slice(512 * n, 512 * n + 512)
        for m in range(4):
            pq = ps_x.tile([128, 512], F32, tag="px")
            for o in range(NO):
                nc.tensor.matmul(
                    pq[:], lhsT=wa[:, o, 128 * m:128 * m + 128],
                    rhs=xt[:, o, n_sl],
                    start=(o == 0), stop=(o == NO - 1),
                    skip_group_check=True)
                yield
            dst = qt[:, m, n_sl] if m < 2 else kt[:, m - 2, n_sl]
            nc.scalar.copy(dst, pq[:])
            yield

    def gen_qkm2(m, n1, n2):
        # QK chunk m for TWO token blocks sharing each wa stationary, so
        # the per-matmul weight swap (which serializes against the
        # in-flight matmul's drain on full-K loads) amortizes over 2
        # matmuls.  Holds both px ring slots for the o-loop.
        sl1 = slice(512 * n1, 512 * n1 + 512)
        sl2 = slice(512 * n2, 512 * n2 + 512)
        pqa = ps_x.tile([128, 512], F32, tag="px")
        pqb = ps_x.tile([128, 512], F32, tag="px")
        for o in range(NO):
            lhsT = wa[:, o, 128 * m:128 * m + 128]
            nc.tensor.matmul(pqa[:], lhsT=lhsT, rhs=xt[:, o, sl1],
                             start=(o == 0), stop=(o == NO - 1),
                             skip_group_check=True)
            yield
            nc.tensor.matmul(pqb[:], lhsT=lhsT, rhs=xt[:, o, sl2],
                             start=(o == 0), stop=(o == NO - 1),
                             skip_group_check=True)
            yield
        if m < 2:
            nc.scalar.copy(qt[:, m, sl1], pqa[:])
            yield
            nc.scalar.copy(qt[:, m, sl2], pqb[:])
        else:
            nc.scalar.copy(kt[:, m - 2, sl1], pqa[:])
            yield
            nc.scalar.copy(kt[:, m - 2, sl2], pqb[:])
        yield

    def gen_v(n):
        # v chunks 4n..4n+3 (tokens of block n); casts on DVE.
        for t in range(4 * n, 4 * n + 4):
            pv = ps_x.tile([128, 512], F32, tag="px")
            for o in range(NO):
                nc.tensor.matmul(
                    pv[:, :DHC], lhsT=xt[:, o, 128 * t:128 * t + 128],
                    rhs=wa[:, o, 2 * DHC:3 * DHC],
                    start=(o == 0), stop=(o == NO - 1),
                    skip_group_check=True)
                yield
            pv4 = pv[:, :DHC].rearrange("p (h c) -> p h c", c=DH)
            # even heads -> cols 0:64, odd heads -> cols 64:128
            nc.vector.tensor_copy(vsb[:, t, 0::2, 0:DH], pv4[:, 0::2, :])
            nc.vector.tensor_copy(vsb[:, t, 1::2, DH:128], pv4[:, 1::2, :])
            yield

    # ---- out projection for one token block ----
    outT_m = outT.rearrange("(mm p) s -> mm p s", p=128)

    def gen_op(n):
        n_sl = slice(512 * n, 512 * n + 512)
        for m in range(NO):
            po = ps_x.tile([128, 512], F32, tag="px")
            for p in (0, 1):
                nc.tensor.matmul(
                    po[:], lhsT=wos[:, p, 128 * m:128 * m + 128],
                    rhs=ctxT[:, p, n_sl],
                    start=(p == 0), stop=(p == 1), skip_group_check=True)
                yield
            osb = outp.tile([128, 512], BF, tag="osb")
            nc.vector.tensor_copy(osb[:], po[:])
            nc.sync.dma_start(outT_m[m, :, n_sl], osb[:])
            yield

    # ---- attention window (pair p, q-window j) ----
    def window(p, j, stream, per_i):
        n_i = 4 * j + 4
        q_sl = slice(512 * j, 512 * j + 512)
        pcA = ps_c.tile([128, 512], F32, tag="pc")
        pcB = ps_c.tile([128, 512], F32, tag="pc")

        def emit_scores(i):
            d = i - 4 * j
            # diagonal tiles: k-chunk i only reaches q >= 128*d in this
            # q-window; restrict all work to the valid column range.
            q0 = 128 * d if d > 0 else 0
            k_sl = slice(128 * i, 128 * i + 128)
            qv_sl = slice(512 * j + q0, 512 * j + 512)
            pss = ps_s.tile([128, 1024], F32, tag="ps")
            # two heads as concurrent row-tiled K=64 matmuls
            nc.tensor.matmul(pss[:, q0:512],
                             lhsT=kt[0:64, p, k_sl], rhs=qt[0:64, p, qv_sl],
                             start=True, stop=True)
            nc.tensor.matmul(pss[:, 512 + q0:1024],
                             lhsT=kt[64:128, p, k_sl],
                             rhs=qt[64:128, p, qv_sl],
                             start=True, stop=True)
            return pss, q0

        # software pipeline: scores for i+1 are emitted before ctx of i so
        # the PE never sits behind a wait on the exp of i.
        pss_cur, q0_cur = emit_scores(0)
        for i in range(n_i):
            q0 = q0_cur
            eT = work.tile([128, 2, 512], BF, tag="exp")
            pv2 = pss_cur.rearrange("p (g f) -> p g f", g=2)
            nc.scalar.activation(eT[:, :, q0:512], pv2[:, :, q0:512],
                                 EXPF, scale=0.125)
            if i + 1 < n_i:
                pss_cur, q0_cur = emit_scores(i + 1)
            stream.take(per_i)
            if i - 4 * j >= 0:  # triangular 128x128 mask on the diagonal
                for h in (0, 1):
                    nc.gpsimd.tensor_mul(eT[:, h, q0:q0 + 128],
                                         eT[:, h, q0:q0 + 128], msk[:])
            # ctx accumulation; ones columns accumulate denominators
            nc.tensor.matmul(
                pcA[0:65, q0:512], lhsT=vsb[:, i, 2 * p, 0:65],
                rhs=eT[:, 0, q0:512],
                start=(i == 0), stop=(i == n_i - 1), skip_group_check=True)
            nc.tensor.matmul(
                pcB[:, q0:512], lhsT=vsb[:, i, 2 * p + 1, :],
                rhs=eT[:, 1, q0:512],
                start=(i == 0), stop=(i == n_i - 1), skip_group_check=True)

        # ---- immediate PSUM evacuation: two merged casts free the ctx
        # banks ~1.2us after the last ctx matmul (den rows ride along).
        scr = scrp.tile([128, 1024], BF, tag="scr")
        nc.vector.tensor_copy(scr[0:65, 0:512], pcA[0:65, :])
        nc.vector.tensor_copy(scr[32:33, 512:1024], pcB[32:33, :])
        nc.vector.tensor_copy(scr[64:128, 512:1024], pcB[64:128, :])

        def tail():
            # deferred into the next window: den broadcast (K=1
            # ones-matmuls), reciprocal, scr -> ctxT normalize muls.
            # (reciprocal/partition-broadcast must run from partition 0 on
            # HW, hence the matmul broadcast.)
            pd = ps_x.tile([128, 512], F32, tag="px")
            nc.tensor.matmul(pd[0:64, :], lhsT=ones[64:65, :],
                             rhs=scr[64:65, 0:512],
                             start=True, stop=True, tile_position=(64, 0))
            yield
            nc.tensor.matmul(pd[64:128, :], lhsT=ones[32:33, :],
                             rhs=scr[32:33, 512:1024],
                             start=True, stop=True, tile_position=(32, 64))
            yield
            rec = recp.tile([128, 512], F32, tag="rec")
            nc.vector.reciprocal_approx_fast(out=rec[:, :], in_=pd[:, :])
            nc.vector.tensor_mul(ctxT[0:64, p, q_sl], scr[0:64, 0:512],
                                 rec[0:64, :])
            nc.vector.tensor_mul(ctxT[64:128, p, q_sl],
                                 scr[64:128, 512:1024], rec[64:128, :])
            yield

        return tail()

    # ---- orchestration ----
    # Tails are pushed to the BACK of the stream: they contain a px-pool
    # tile (pd), and interjecting mid-generator would deadlock the px ring
    # while a gen_qkm2 holds both slots.  scrp bufs=3 gives the deferred
    # tails the needed slack.
    qk_block0()
    st = Stream()
    st.push(gen_v(0))
    st.drain()                       # V(0) dense in the prologue

    st.push(gen_v(1))
    for m in range(4):
        st.push(gen_qkm2(m, 1, 2))
    st.push(window(0, 0, st, per_i=14))
    st.push(window(1, 0, st, per_i=14))
    st.push(gen_op(0))
    st.push(gen_qk(3))
    st.push(gen_v(2))
    st.push(window(0, 1, st, per_i=10))
    st.push(window(1, 1, st, per_i=10))
    st.push(gen_op(1))
    st.push(window(0, 2, st, per_i=4))
    st.push(window(1, 2, st, per_i=4))
    st.push(gen_v(3))
    st.push(gen_op(2))
    st.push(window(0, 3, st, per_i=3))
    st.push(window(1, 3, st, per_i=3))
    st.drain()
    for _ in gen_op(3):
        pass
    ctx.close()


def _get_nc():
    global _NC_CACHE
    if _NC_CACHE is None:
        _NC_CACHE = _build_core_kernel()
    return _NC_CACHE


def _build_masks():
    p = np.arange(128)[:, None]
    f = np.arange(128)[None, :]
    return (p <= f).astype(BF16)


def _shard_inputs(x, Wq, Wk, Wv, Wo):
    xb = x.astype(BF16)
    masks = _build_masks()
    in_maps = []
    for c in range(N_CORES):
        b, g = divmod(c, 4)
        cols = slice(DHC * g, DHC * g + DHC)
        w_all = np.ascontiguousarray(np.concatenate(
            [Wq[:, cols], Wk[:, cols], Wv[:, cols]], axis=1).astype(BF16))
        wo_s = np.ascontiguousarray(Wo[cols, :].astype(BF16))
        xT = np.ascontiguousarray(xb[b].T)
        in_maps.append({"xT": xT, "w_all": w_all, "wo": wo_s, "masks": masks})
    return in_maps


def _unshard(results, bo):
    out = np.empty((2, S, D), np.float32)
    for b in range(2):
        acc = results[4 * b]["outT"].astype(np.float32)
        for g in range(1, 4):
            acc += results[4 * b + g]["outT"].astype(np.float32)
        out[b] = acc.T + bo.astype(np.float32)
    return out


def run(x, Wq, Wk, Wv, Wo, bo, trace=False, **spmd_kwargs):
    nc = _get_nc()
    in_maps = _shard_inputs(x, Wq, Wk, Wv, Wo)
    res = bass_utils.run_bass_kernel_spmd(
        nc, in_maps, core_ids=list(range(N_CORES)), trace=trace,
        **spmd_kwargs)
    return _unshard(res.results, bo), res


def kernel(x, Wq, Wk, Wv, Wo, bo):
    out, _ = run(np.asarray(x), np.asarray(Wq), np.asarray(Wk),
                 np.asarray(Wv), np.asarray(Wo), np.asarray(bo))
    return out
